# revision 12
# baseline (speedup 1.0000x reference)
"""DistancePenaltyLoss Trainium2 kernel (8-core SPMD, full-input contract).

Strategy
--------
loss = mean_i [ lse_i - x[i,t_i] + sum_j probs[i,j] * M[t_i, j] ]
with M = node_D + area_D[n2a[:,None], n2a[None,:]] (22x22, host-combined),
lse_i = log sum_j exp(x[i,j]), probs = exp(x)/s (no max-subtraction needed:
logits ~ N(0,1), exp cannot overflow fp32).

Host sorts rows by target class and shards them across 8 cores so that every
128-row "group" is single-class and the group->class map is identical on all
cores (one SPMD program; structure is data-dependent, compiled per class
histogram and memoized). On device, per batch of up to 23 groups of class k:
  PSUM region[k] += r_batch^T E_batch      (r = 1/rowsum, E = exp(logits))
giving, on the diagonal blocks, S[k,:] = sum_{t_i=k} probs[i,:]. The CE
gather sum_i x[i,t_i] becomes per-class-run column sums of the logits
(strided DVE reduces), and lse is accumulated by ScalarE (Ln + accum_out).
The final 22x22 reduction, CE assembly, and pad-row corrections happen on
host in float64:  pen = <S, M>.
"""

import os
import sys
from contextlib import ExitStack

import numpy as np

for _p in ("/opt/trn_rl_repo", "/root/.axon_site/_ro/trn_rl_repo"):
    if os.path.isdir(_p) and _p not in sys.path:
        sys.path.insert(0, _p)

import concourse.bacc as bacc
import concourse.bass as bass
import concourse.tile as tile
from concourse import mybir
from concourse.bass_utils import run_bass_kernel_spmd

N_CORES = 8
C = 22          # classes
P = 128         # SBUF partitions
GMAX = 23       # groups per matmul batch (23*22 = 506 <= 512 psum bank)
N_CHUNK = 128   # groups per SBUF chunk
N_BANKS = 8
BANK_F32 = 512
OUT_ROWS = 87   # max psum region row = 64 + 23
F32 = mybir.dt.float32
BF16 = mybir.dt.bfloat16

ALPHA, BETA = 1.0, 1.0

_prog_cache: dict = {}
last_run_info: dict = {}


# --------------------------------------------------------------------------- #
# host-side prep
# --------------------------------------------------------------------------- #

def _prep(logits, targets):
    """Sort rows by class, split across cores with an identical group map.

    Returns (shards [P, n_total, C] f32 per core, segments [(k, g0, Gk)],
    n_total, pad_counts [N_CORES, C])."""
    t = np.asarray(targets).astype(np.int64).ravel()
    logits = np.ascontiguousarray(np.asarray(logits, dtype=np.float32))
    order = np.argsort(t, kind="stable")
    cnt = np.bincount(t, minlength=C)
    base = cnt // N_CORES
    rem = cnt % N_CORES
    maxrows = base + (rem > 0).astype(np.int64)
    G = -(-maxrows // P)  # ceil; 0 for empty classes
    n_total = int(G.sum())
    segments = []
    g = 0
    for k in range(C):
        if G[k] > 0:
            segments.append((k, g, int(G[k])))
            g += int(G[k])
    cls_off = np.concatenate([[0], np.cumsum(cnt)])

    shards = []
    pad_counts = np.zeros((N_CORES, C), np.int64)
    for j in range(N_CORES):
        rows = np.full(n_total * P, -1, dtype=np.int64)
        for (k, g0, Gk) in segments:
            nkj = int(base[k] + (1 if j < rem[k] else 0))
            s = int(cls_off[k] + j * base[k] + min(j, int(rem[k])))
            rows[g0 * P : g0 * P + nkj] = order[s : s + nkj]
            pad_counts[j, k] = Gk * P - nkj
        arr = np.zeros((n_total * P, C), np.float32)
        valid = rows >= 0
        arr[valid] = logits[rows[valid]]
        # group-major -> partition-major: dram[p, g, :] = row (g*128 + p)
        arr = np.ascontiguousarray(arr.reshape(n_total, P, C).transpose(1, 0, 2))
        shards.append(arr)
    return shards, segments, n_total, pad_counts


def _batches(segments, n_total):
    """Matmul batches: class segments clipped at chunk boundaries, <=GMAX."""
    n_chunks = -(-n_total // N_CHUNK)
    per_chunk = [[] for _ in range(n_chunks)]
    for (k, g0, Gk) in segments:
        b0 = g0
        end = g0 + Gk
        while b0 < end:
            ci = b0 // N_CHUNK
            bg = min(GMAX, end - b0, (ci + 1) * N_CHUNK - b0)
            per_chunk[ci].append((k, b0, bg))
            b0 += bg
    return per_chunk


def _runs(segments, n_total):
    """CE runs: class segments clipped at chunk boundaries only."""
    n_chunks = -(-n_total // N_CHUNK)
    per_chunk = [[] for _ in range(n_chunks)]
    ri = 0
    for (k, g0, Gk) in segments:
        b0 = g0
        end = g0 + Gk
        while b0 < end:
            ci = b0 // N_CHUNK
            ln = min(end - b0, (ci + 1) * N_CHUNK - b0)
            per_chunk[ci].append((ri, k, b0, ln))
            ri += 1
            b0 += ln
    return per_chunk, ri


def _region(k):
    return 32 * (k % 3), k // 3  # (psum partition base, bank)


# --------------------------------------------------------------------------- #
# device program
# --------------------------------------------------------------------------- #

def _build_program(n_total, segments):
    nc = bacc.Bacc("TRN2", target_bir_lowering=False, debug=False, num_devices=N_CORES)
    per_chunk = _batches(segments, n_total)
    run_chunk, nruns = _runs(segments, n_total)
    n_chunks = len(per_chunk)
    L_d = nc.dram_tensor("logits_sh", [P, n_total, C], F32, kind="ExternalInput")
    O_d = nc.dram_tensor("out_psum", [OUT_ROWS, N_BANKS, BANK_F32], F32, kind="ExternalOutput")
    S_d = nc.dram_tensor("out_lse", [P, 1], F32, kind="ExternalOutput")
    E_d = nc.dram_tensor("out_ce", [P, nruns], F32, kind="ExternalOutput")

    with ExitStack() as ctx:
        tc = ctx.enter_context(tile.TileContext(nc))
        lp = ctx.enter_context(tc.tile_pool(name="lp", bufs=4))
        ep = ctx.enter_context(tc.tile_pool(name="ep", bufs=4))
        rp = ctx.enter_context(tc.tile_pool(name="rp", bufs=4))
        pp = ctx.enter_context(tc.tile_pool(name="pp", bufs=1))
        ps = ctx.enter_context(
            tc.tile_pool(name="ps", bufs=1, space=bass.MemorySpace.PSUM)
        )

        Pt = ps.tile([P, N_BANKS, BANK_F32], F32)
        s_all = pp.tile([P, n_total], BF16)
        ls = pp.tile([P, n_total], BF16)
        ce_runs = pp.tile([P, nruns], F32)
        zw = pp.tile([P, OUT_ROWS], F32)
        zs = pp.tile([P, BANK_F32], F32)
        acc = pp.tile([P, 1], F32)

        nc.vector.memset(zw[:], 0.0)
        nc.gpsimd.memset(zs[:], 0.0)
        # Zero the used PSUM rows with start=True matmuls (has_written-safe
        # across re-runs).
        for b in range(N_BANKS):
            nc.tensor.matmul(
                Pt[0:OUT_ROWS, b, :],
                zw[:],
                zs[:],
                start=True,
                stop=True,
                skip_group_check=True,
            )

        for ci in range(n_chunks):
            g0 = ci * N_CHUNK
            gn = min(N_CHUNK, n_total - g0)
            Lt = lp.tile([P, N_CHUNK, C], F32)
            nc.sync.dma_start(Lt[:, :gn, :], L_d[:, g0 : g0 + gn, :])
            Et = ep.tile([P, N_CHUNK, C], BF16)
            nc.scalar.activation(
                Et[:, :gn, :], Lt[:, :gn, :], mybir.ActivationFunctionType.Exp
            )
            with nc.allow_low_precision("bf16 rowsum: 22-elem sums, fp32 internal accum"):
                nc.vector.reduce_sum(
                    s_all[:, g0 : g0 + gn], Et[:, :gn, :], axis=mybir.AxisListType.X
                )
            Rt = rp.tile([P, N_CHUNK], BF16)
            with nc.allow_low_precision("bf16 reciprocal feeding prob sums"):
                nc.vector.reciprocal(Rt[:, :gn], s_all[:, g0 : g0 + gn])
            for (k, b0, bg) in per_chunk[ci]:
                off = b0 - g0
                p0, bk = _region(k)
                nc.tensor.matmul(
                    Pt[p0 : p0 + bg, bk, 0 : C * bg],
                    Rt[:, off : off + bg],
                    Et[:, off : off + bg, :],
                    start=False,
                    stop=False,
                    skip_group_check=True,
                )
            for (ri, k, b0, ln) in run_chunk[ci]:
                off = b0 - g0
                nc.vector.reduce_sum(
                    ce_runs[:, ri : ri + 1],
                    Lt[:, off : off + ln, k],
                    axis=mybir.AxisListType.X,
                )

        nc.scalar.activation(
            ls[:], s_all[:], mybir.ActivationFunctionType.Ln, accum_out=acc[:]
        )
        out_sb = pp.tile([P, N_BANKS, BANK_F32], F32)
        nc.scalar.copy(out_sb[0:OUT_ROWS], Pt[0:OUT_ROWS])
        nc.sync.dma_start(S_d[:], acc[:])
        nc.sync.dma_start(E_d[:], ce_runs[:])
        nc.sync.dma_start(O_d[:], out_sb[0:OUT_ROWS])
    nc.compile()
    return nc


# --------------------------------------------------------------------------- #
# host-side combine
# --------------------------------------------------------------------------- #

def _combine(psums, accs, ces, segments, pad_counts, M2, B):
    lse_sum = float(sum(a.sum(dtype=np.float64) for a in accs))
    ce_gather = float(sum(cr.sum(dtype=np.float64) for cr in ces))
    V = np.zeros((C, C), np.float64)
    ii = np.arange(GMAX)
    cols = (C * ii)[:, None] + np.arange(C)[None, :]  # [GMAX, C] diag-block cols
    for ps_arr in psums:
        for (k, _g0, _Gk) in segments:
            p0, bk = _region(k)
            reg = ps_arr[p0 : p0 + GMAX, bk, : C * GMAX].astype(np.float64)
            V[k] += np.take_along_axis(reg, cols, axis=1).sum(axis=0)
    import ml_dtypes

    # Device pad rows: e = bf16(exp(0)) = 1, s = bf16(22) = 22, r = bf16(1/22).
    r_pad = float(np.asarray(1.0 / np.float32(22.0)).astype(ml_dtypes.bfloat16))
    pad_k = pad_counts.sum(axis=0).astype(np.float64)
    lse_sum -= float(pad_k.sum()) * float(np.log(22.0))
    pen = float((V * M2).sum()) - float((pad_k * (M2.sum(axis=1) * r_pad)).sum())
    return (lse_sum - ce_gather + pen) / B


# --------------------------------------------------------------------------- #
# entry point
# --------------------------------------------------------------------------- #

def kernel(logits, targets, node_distance_matrix, area_distance_matrix, node_to_area):
    B = int(np.asarray(logits).shape[0])
    n2a = np.asarray(node_to_area).astype(np.int64).ravel()
    M2 = ALPHA * np.asarray(node_distance_matrix, np.float64) + BETA * np.asarray(
        area_distance_matrix, np.float64
    )[n2a[:, None], n2a[None, :]]

    shards, segments, n_total, pad_counts = _prep(logits, targets)

    key = (n_total, tuple(segments))
    nc = _prog_cache.get(key)
    if nc is None:
        nc = _build_program(n_total, segments)
        _prog_cache[key] = nc

    in_maps = [{"logits_sh": sh} for sh in shards]
    trace = bool(int(os.environ.get("KERNEL_TRACE", "0")))
    res = run_bass_kernel_spmd(nc, in_maps, list(range(N_CORES)), trace=trace)
    last_run_info["exec_time_ns"] = res.exec_time_ns
    last_run_info["results"] = res

    psums = [r["out_psum"] for r in res.results]
    accs = [r["out_lse"] for r in res.results]
    ces = [r["out_ce"] for r in res.results]
    loss = _combine(psums, accs, ces, segments, pad_counts, M2, B)
    return np.float32(loss)
